# revision 23
# baseline (speedup 1.0000x reference)
"""Multi-head attention with interleaved RoPE on 8 Trainium2 NeuronCores.

Strategy: tensor-parallel over heads. Each core owns 2 of the 16 heads:
  - Q/K/V weights column-sliced (256 cols/core), out_proj row-sliced.
  - Each core computes its heads' attention and a partial out-projection;
    the host sums the 8 partials (plus the bias term bv@Wo + bo folded
    out of the device program entirely -- softmax rows sum to 1, so the
    v-bias contributes exactly bv@Wo to every output row).

Device dataflow (per core; fp16 operands, fp32 psum, fp8 QK-proj):
  qT/kT = Wq8.T @ xT8 via fp8 DoubleRow (K=256/matmul), descaled +bias
          on ScalarE, RoPE on VectorE          [d_head, tok] per head
  v = xT.T @ Wv (fp16)                         [tok, dv]
  per (j-chunk of 512 q, i-tile of 128 keys):
    pl = [kT_h.T @ qT_h for h in 2]            [keys, 1024] psum
    ex = exp(pl/128) one 1024-wide activation  [keys, 1024] fp16
    po_h += v_h.T @ ex_h                       [dv, 512] psum accum
    ps2 += ones128.T @ ex                      [128, 1024] psum accum
  rr = recip(ps2); ahat_h = po_h * rr_h        (fused psum-mul on DVE)
  partial = sum_h ahat_h.T @ Wo_h rows         [tok, D] -> DMA out fp16
"""

import os

import numpy as np

B = 2
N = 2048  # tokens per batch
D = 2048  # model dim
H = 16
HD = 128  # head dim
NCORES = 8
HPC = H // NCORES  # heads per core = 2
DLOC = HPC * HD  # local width = 256
DC = D // 128  # contraction chunks = 16
NT = N // 128  # token tiles per batch = 16

MM_DT_NAME = os.environ.get("ATTN_MM_DT", "float16")
QK_FP8 = os.environ.get("ATTN_QK_FP8", "1") == "1"
SUM_FP8 = os.environ.get("ATTN_SUM_FP8", "1") == "1"
W_SCALE = 64.0  # fp8 weight pre-scale (host) folded out in the bias-add

_COMPILED = {}


def _build_nc():
    import concourse.bacc as bacc
    import concourse.mybir as mybir
    import concourse.tile as tile

    f32 = mybir.dt.float32
    f8 = mybir.dt.float8e4
    sd = getattr(mybir.dt, MM_DT_NAME)
    DR = mybir.MatmulPerfMode.DoubleRow

    nc = bacc.Bacc("TRN2", target_bir_lowering=False, debug=False,
                   num_devices=NCORES)

    x_in = nc.dram_tensor("x", [B, DC, 128, N], sd, kind="ExternalInput").ap()
    w8dt = f8 if QK_FP8 else sd
    if QK_FP8:
        x8_in = nc.dram_tensor("x8", [B, DC, 128, N], f8,
                               kind="ExternalInput").ap()
    # weights arrive pre-rearranged: partition-major, contiguous DMA lines
    wq_in = nc.dram_tensor("wq", [128, DC, DLOC], w8dt,
                           kind="ExternalInput").ap()
    wk_in = nc.dram_tensor("wk", [128, DC, DLOC], w8dt,
                           kind="ExternalInput").ap()
    wv_in = nc.dram_tensor("wv", [128, DC, DLOC], sd,
                           kind="ExternalInput").ap()
    wo_in = nc.dram_tensor("wo", [128, HPC, D], sd,
                           kind="ExternalInput").ap()
    bq_in = nc.dram_tensor("bq", [HPC, 128, 1], f32, kind="ExternalInput").ap()
    bk_in = nc.dram_tensor("bk", [HPC, 128, 1], f32, kind="ExternalInput").ap()
    cos_in = nc.dram_tensor("cosT", [HD, N], sd, kind="ExternalInput").ap()
    s2_in = nc.dram_tensor("s2T", [HD, N], sd, kind="ExternalInput").ap()
    ones8_in = nc.dram_tensor("ones8", [128, 2, 128], f8,
                              kind="ExternalInput").ap()
    out_p = nc.dram_tensor("out_p", [B, N, D], sd, kind="ExternalOutput").ap()

    Exp = mybir.ActivationFunctionType.Exp
    Ident = mybir.ActivationFunctionType.Identity
    inv_d = 1.0 / HD  # folds the module's two 1/sqrt(d) logit scalings
    qk_descale = (1.0 / W_SCALE) if QK_FP8 else 1.0

    with tile.TileContext(nc) as tc:
        with (
            tc.tile_pool(name="persist", bufs=1) as pers,
            tc.tile_pool(name="ps", bufs=2, space="PSUM") as ps_pool,
            tc.tile_pool(name="pexp", bufs=3) as pexp_pool,
            tc.tile_pool(name="prope", bufs=4) as prope_pool,
            tc.tile_pool(name="pout", bufs=3) as pout_pool,
            tc.tile_pool(name="prr", bufs=2) as prr_pool,
        ):
            # ---- persistent SBUF tensors ---------------------------------
            # fp8 x first: QK-proj can start after ~4MB instead of ~12MB
            wq_sb = pers.tile([128, DC, DLOC], w8dt, tag="wq_sb")
            wk_sb = pers.tile([128, DC, DLOC], w8dt, tag="wk_sb")
            if QK_FP8:
                # chunk-major (contiguous 2KB lines); sync+gpsimd in parallel
                xT8 = pers.tile([128, DC, N], f8, tag="xT8")
                for dq in range(4):
                    e = nc.sync if dq % 2 == 0 else nc.gpsimd
                    e.dma_start(
                        out=xT8[:, dq * 4 : (dq + 1) * 4, :],
                        in_=x8_in[0, dq * 4 : (dq + 1) * 4].rearrange(
                            "a p t -> p a t"))
            nc.scalar.dma_start(wq_sb, wq_in)
            nc.scalar.dma_start(wk_sb, wk_in)
            cos_sb = pers.tile([HD, N], sd, tag="cos_sb")
            s2_sb = pers.tile([HD, N], sd, tag="s2_sb")
            nc.scalar.dma_start(cos_sb, cos_in)
            nc.scalar.dma_start(s2_sb, s2_in)
            bq_sb = pers.tile([128, HPC], f32, tag="bq_sb")
            bk_sb = pers.tile([128, HPC], f32, tag="bk_sb")
            for h in range(HPC):
                nc.scalar.dma_start(bq_sb[:, h : h + 1], bq_in[h])
                nc.scalar.dma_start(bk_sb[:, h : h + 1], bk_in[h])

            zb = pers.tile([128, 1], f32, tag="zb")
            nc.vector.memset(zb, 0.0)
            # preload the exp table-set during the initial DMA wait
            dmy = pers.tile([128, 1], f32, tag="dmy")
            nc.scalar.activation(dmy, zb, Exp, bias=zb, scale=1.0)
            if SUM_FP8:
                ones8 = pers.tile([128, 2, 128], f8, tag="ones8")
                nc.scalar.dma_start(ones8, ones8_in)
            else:
                ones128 = pers.tile([128, 128], sd, tag="ones128")
                nc.vector.memset(ones128, 1.0)
            # dummy matmuls: keep the PE clock warm (HAM) while the initial
            # x8 DMAs land (~12us of otherwise-idle PE)
            warm = pers.tile([128, 128], sd, tag="warm")
            nc.vector.memset(warm, 0.0)
            for _ in range(110):
                pw = ps_pool.tile([128, 128], f32, tag="big", bufs=2)
                nc.tensor.matmul(pw, warm, warm, start=True, stop=True)

            xT = pers.tile([128, DC, N], sd, tag="xT")
            for dq in range(4):
                e = nc.sync if dq % 2 == 0 else nc.gpsimd
                e.dma_start(out=xT[:, dq * 4 : (dq + 1) * 4, :],
                            in_=x_in[0, dq * 4 : (dq + 1) * 4].rearrange(
                                "a p t -> p a t"))
            wv_sb = pers.tile([128, DC, DLOC], sd, tag="wv_sb")
            nc.sync.dma_start(wv_sb, wv_in)
            wo_sb = pers.tile([128, HPC, D], sd, tag="wo_sb")
            nc.gpsimd.dma_start(wo_sb, wo_in)

            qT = pers.tile([128, HPC, N], sd, tag="qT")
            kT = pers.tile([128, HPC, N], sd, tag="kT")
            v_sb = pers.tile([128, NT, DLOC], sd, tag="v_sb")
            ahat = pers.tile([128, HPC, N], sd, tag="ahat")

            # swap even/odd partitions within each 32-lane quadrant (RoPE)
            swap_mask = [i + 1 if i % 2 == 0 else i - 1 for i in range(32)]

            for b in range(B):
                # ======== load pre-transposed x for this batch ============
                nc.enter_named_scope(f"xload{b}", False)
                if b > 0:
                    if QK_FP8:
                        for dq in range(4):
                            e = nc.sync if dq % 2 == 0 else nc.gpsimd
                            e.dma_start(
                                out=xT8[:, dq * 4 : (dq + 1) * 4, :],
                                in_=x8_in[b, dq * 4 : (dq + 1) * 4].rearrange(
                                    "a p t -> p a t"))
                    for dq in range(4):
                        e = nc.sync if dq % 2 == 0 else nc.gpsimd
                        e.dma_start(
                            out=xT[:, dq * 4 : (dq + 1) * 4, :],
                            in_=x_in[b, dq * 4 : (dq + 1) * 4].rearrange(
                                "a p t -> p a t"))
                nc.leave_named_scope(f"xload{b}", None, False)
                # ======== Q/K projections + RoPE ==========================
                nc.enter_named_scope(f"proj{b}", False)
                for nch in range(N // 512):
                    for wsb, bsb, dst in ((wq_sb, bq_sb, qT),
                                          (wk_sb, bk_sb, kT)):
                        for h in range(HPC):
                            tok = slice(nch * 512, (nch + 1) * 512)
                            pq = ps_pool.tile([128, 512], f32, tag="big",
                                              bufs=2)
                            if QK_FP8:
                                for c in range(DC // 2):
                                    nc.tensor.matmul(
                                        pq,
                                        wsb[:, 2 * c : 2 * c + 2,
                                            h * 128 : (h + 1) * 128],
                                        xT8[:, 2 * c : 2 * c + 2, tok],
                                        start=(c == 0),
                                        stop=(c == DC // 2 - 1),
                                        perf_mode=DR,
                                    )
                            else:
                                for dc in range(DC):
                                    nc.tensor.matmul(
                                        pq,
                                        wsb[:, dc, h * 128 : (h + 1) * 128],
                                        xT[:, dc, tok],
                                        start=(dc == 0),
                                        stop=(dc == DC - 1),
                                    )
                            # descale + bias on ScalarE (idle in this phase)
                            nc.scalar.activation(dst[:, h, tok], pq, Ident,
                                                 bias=bsb[:, h : h + 1],
                                                 scale=qk_descale)
                            # RoPE on VectorE, overlapped with PE
                            src = dst[:, h, tok]
                            sw = prope_pool.tile([128, 512], sd, tag="sw")
                            tm = prope_pool.tile([128, 512], sd, tag="tm")
                            nc.vector.stream_shuffle(sw, src, swap_mask)
                            nc.vector.tensor_mul(tm, src, cos_sb[:, tok])
                            nc.vector.tensor_mul(sw, sw, s2_sb[:, tok])
                            nc.vector.tensor_add(src, tm, sw)
                # ======== V projection (fp16) =============================
                for tt in range(NT):
                    pv = ps_pool.tile([128, DLOC], f32, tag="big", bufs=2)
                    for dc in range(DC):
                        nc.tensor.matmul(
                            pv,
                            xT[:, dc, tt * 128 : (tt + 1) * 128],
                            wv_sb[:, dc, :],
                            start=(dc == 0),
                            stop=(dc == DC - 1),
                        )
                    nc.scalar.copy(v_sb[:, tt, :], pv)
                nc.leave_named_scope(f"proj{b}", None, False)
                # ======== attention, per 512-q-chunk ======================
                nc.enter_named_scope(f"attn{b}", False)
                for j in range(N // 512):
                    jq = slice(j * 512, (j + 1) * 512)
                    po = [ps_pool.tile([128, 512], f32, tag="po", bufs=2,
                                       name=f"po{h}") for h in range(HPC)]
                    ps2 = ps_pool.tile([128, 1024], f32, tag="ps2", bufs=1,
                                       name="ps2")

                    # software-pipelined: logits/exp for i+1 are emitted
                    # BEFORE AV/sums of i so the PE never waits on exp's
                    # cross-engine round-trip
                    def logits_exp(i):
                        pl = ps_pool.tile([128, 1024], f32, tag="big",
                                          bufs=2, name="pl")
                        for h in range(HPC):
                            nc.tensor.matmul(
                                pl[:, h * 512 : (h + 1) * 512],
                                kT[:, h, i * 128 : (i + 1) * 128],
                                qT[:, h, jq],
                                start=True, stop=True,
                            )
                        ex = pexp_pool.tile([128, 1024], sd, tag="ex",
                                            name="ex")
                        nc.scalar.activation(ex, pl, Exp, bias=zb,
                                             scale=inv_d)
                        return ex

                    ex_cur = logits_exp(0)
                    ex8 = None
                    for i in range(NT):
                        ex_nxt = logits_exp(i + 1) if i + 1 < NT else None
                        # cast ex to fp8 pairs (DVE) for DoubleRow sums:
                        # denominator errors average out over 2048 keys
                        if SUM_FP8:
                            if i % 2 == 0:
                                ex8 = [pexp_pool.tile([128, 2, 512], f8,
                                                      tag=f"ex8_{h}", bufs=2,
                                                      name=f"ex8_{h}")
                                       for h in range(HPC)]
                            for h in range(HPC):
                                nc.vector.tensor_copy(
                                    ex8[h][:, i % 2, :],
                                    ex_cur[:, h * 512 : (h + 1) * 512])
                        for h in range(HPC):
                            nc.tensor.matmul(
                                po[h],
                                v_sb[:, i, h * 128 : (h + 1) * 128],
                                ex_cur[:, h * 512 : (h + 1) * 512],
                                start=(i == 0), stop=(i == NT - 1),
                            )
                        if SUM_FP8:
                            if i % 2 == 1:
                                for h in range(HPC):
                                    nc.tensor.matmul(
                                        ps2[:, h * 512 : (h + 1) * 512],
                                        ones8,
                                        ex8[h],
                                        start=(i == 1), stop=(i == NT - 1),
                                        perf_mode=DR,
                                    )
                        else:
                            for h in range(HPC):
                                nc.tensor.matmul(
                                    ps2[:, h * 512 : (h + 1) * 512],
                                    ones128,
                                    ex_cur[:, h * 512 : (h + 1) * 512],
                                    start=(i == 0), stop=(i == NT - 1),
                                )
                        ex_cur = ex_nxt
                    # normalize: rr = 1/ps2, ahat = po * rr (fused psum mul)
                    rr = prr_pool.tile([128, 1024], f32, tag="rr")
                    nc.vector.reciprocal_approx_fast(rr, ps2)
                    for h in range(HPC):
                        nc.vector.tensor_mul(
                            ahat[:, h, jq], po[h],
                            rr[:, h * 512 : (h + 1) * 512])
                # ======== out-projection for the whole batch ==============
                for tt in range(NT):
                    trow = slice(tt * 128, (tt + 1) * 128)
                    for nn in range(2):
                        pp = ps_pool.tile([128, 1024], f32, tag="big", bufs=2)
                        for n2 in range(2):
                            col = (nn * 2 + n2) * 512
                            for h in range(HPC):
                                nc.tensor.matmul(
                                    pp[:, n2 * 512 : (n2 + 1) * 512],
                                    ahat[:, h, trow],
                                    wo_sb[:, h, col : col + 512],
                                    start=(h == 0), stop=(h == HPC - 1),
                                )
                        ob = pout_pool.tile([128, 1024], sd, tag="ob")
                        if nn % 2 == 0:
                            nc.vector.tensor_copy(ob, pp)
                        else:
                            nc.scalar.copy(ob, pp)
                        oe = nc.sync if nn % 2 == 0 else nc.gpsimd
                        oe.dma_start(
                            out=out_p[b, trow, nn * 1024 : (nn + 1) * 1024],
                            in_=ob)
                nc.leave_named_scope(f"attn{b}", 0, False)
    nc.compile()
    return nc


def _get_nc():
    if "nc" not in _COMPILED:
        _COMPILED["nc"] = _build_nc()
    return _COMPILED["nc"]


def _rope_tables():
    inv = (1.0 / (np.float32(10000.0)
                  ** (np.arange(0, HD, 2, dtype=np.float32) / np.float32(HD))))
    inv = inv.astype(np.float32)
    t = np.arange(N, dtype=np.float32)
    freqs = t[:, None] * inv[None, :]  # [N, HD/2]
    cosT = np.repeat(np.cos(freqs).astype(np.float32).T, 2, axis=0)  # [HD, N]
    s2T = np.repeat(np.sin(freqs).astype(np.float32).T, 2, axis=0)
    s2T = s2T.copy()
    s2T[0::2, :] *= np.float32(-1.0)
    return np.ascontiguousarray(cosT), np.ascontiguousarray(s2T)


def _make_in_maps(x, Wq, bq, Wk, bk, Wv, Wo):
    import ml_dtypes

    sd = np.float16 if MM_DT_NAME == "float16" else np.float32
    f8 = ml_dtypes.float8_e4m3
    cosT, s2T = _rope_tables()
    cosT = cosT.astype(sd)
    s2T = s2T.astype(sd)
    # pre-transpose x on the host: [B, N, D] -> [B, DC, 128, N]
    xnp = np.asarray(x)
    xt = np.ascontiguousarray(
        xnp.transpose(0, 2, 1).reshape(B, DC, 128, N).astype(sd))
    if QK_FP8:
        xt8 = np.ascontiguousarray(
            xnp.transpose(0, 2, 1).reshape(B, DC, 128, N).astype(f8))
    def _wmaj(w, dt):
        # [D, DLOC] -> partition-major [128, DC, DLOC], contiguous
        return np.ascontiguousarray(
            w.reshape(DC, 128, DLOC).transpose(1, 0, 2)).astype(dt)

    in_maps = []
    for c in range(NCORES):
        cols = slice(c * DLOC, (c + 1) * DLOC)
        m = {
            "x": xt,
            "ones8": np.ones((128, 2, 128), dtype=f8),
            "wv": _wmaj(np.asarray(Wv[:, cols]), sd),
            "wo": np.ascontiguousarray(
                np.asarray(Wo[cols, :]).reshape(HPC, 128, D)
                .transpose(1, 0, 2)).astype(sd),
            "bq": np.ascontiguousarray(bq[cols].reshape(HPC, 128, 1)
                                       .astype(np.float32)),
            "bk": np.ascontiguousarray(bk[cols].reshape(HPC, 128, 1)
                                       .astype(np.float32)),
            "cosT": cosT,
            "s2T": s2T,
        }
        if QK_FP8:
            m["x8"] = xt8
            m["wq"] = _wmaj(np.asarray(Wq[:, cols]) * np.float32(W_SCALE), f8)
            m["wk"] = _wmaj(np.asarray(Wk[:, cols]) * np.float32(W_SCALE), f8)
        else:
            m["wq"] = _wmaj(np.asarray(Wq[:, cols]), sd)
            m["wk"] = _wmaj(np.asarray(Wk[:, cols]), sd)
        in_maps.append(m)
    return in_maps


def run_device(x, Wq, bq, Wk, bk, Wv, bv, Wo, bo, trace=False):
    """Run the 8-core kernel; returns (full_output, BassKernelResults)."""
    from concourse.bass_utils import run_bass_kernel_spmd

    nc = _get_nc()
    in_maps = _make_in_maps(x, Wq, bq, Wk, bk, Wv, Wo)
    res = run_bass_kernel_spmd(nc, in_maps, core_ids=list(range(NCORES)),
                               trace=trace)
    acc = np.zeros((B, N, D), dtype=np.float64)
    for c in range(NCORES):
        acc += res.results[c]["out_p"]
    bias = (bv.astype(np.float64) @ Wo.astype(np.float64)
            + bo.astype(np.float64))
    out = (acc + bias).astype(np.float32)
    return out, res


def kernel(x, Wq, bq, Wk, bk, Wv, bv, Wo, bo):
    out, _ = run_device(x, Wq, bq, Wk, bk, Wv, bv, Wo, bo, trace=False)
    return out
